# revision 9
# baseline (speedup 1.0000x reference)
"""Trainium2 Bass kernel for nn_AlexSNN: 4-layer spiking CNN (conv+BN+LIF) x T=4, mean over T.

Sharding: data-parallel over batch B=16 across 8 cores (2 samples/core).
Per core: all 4 layers stay in SBUF; convs = channels-on-partition shift-matmuls
(L0 via space-to-depth 4x -> 48ch 3x3 stride-1); BN folded into weights host-side;
LIF = 3 fused DVE ops per tile per timestep. Matmuls in float32r (TF32-like, 1cyc/row).
Self-contained: hardcodes all shapes; only needs /opt/trn_rl_repo on sys.path.
"""
import sys
sys.path.insert(0, '/opt/trn_rl_repo')
import numpy as np

TAU = 0.25
EPS = 1e-5
N_CORES = 8
B, T = 16, 4

# layer geometry (out H=W per layer)
H0, H1, H2, H3 = 72, 36, 36, 18
P0 = 76          # plane0 padded (72 + 2*2)
P12 = 38         # plane1/plane2 padded (36 + 2*1)
S2D = 75         # s2d grid (300/4)
NT0, NT12 = 12, 3   # N-tiles: L0 12x432 (6 rows), L1/L2 3x432 (12 rows)

_CACHE = {}


def _fold_bn(w, b, g, be, m, v):
    s = g / np.sqrt(v + EPS)
    wf = (w * s[:, None, None, None]).astype(np.float32)
    bias = (s * (b - m) + be).astype(np.float32)
    return wf, bias


def host_prep(inputs):
    """Fold BN, reorder weights to lhsT layouts, s2d-transform x. Returns (in_maps, meta)."""
    x = inputs['x']
    ws, biases, ths = [], [], []
    for i in range(4):
        wf, bias = _fold_bn(inputs[f'w{i}'], inputs[f'b{i}'], inputs[f'g{i}'],
                            inputs[f'be{i}'], inputs[f'm{i}'], inputs[f'v{i}'])
        ws.append(wf)
        biases.append(bias)
        ths.append(np.asarray(inputs[f'th{i}']))

    # per-channel thresholds (actual data is uniform 0.5; assert-fallback handled by caller)
    th_pc = []
    for th in ths:
        assert np.allclose(th, th[:, :1, :1]), "non-uniform threshold unsupported"
        th_pc.append(th[:, 0, 0].astype(np.float32))
    assert all(np.abs(b).max() < 1e-12 for b in biases), "nonzero conv/BN bias unsupported"

    # L0 weights -> s2d lhsT [3,3,48,64]; row k = c*16 + ry*4 + rx
    w0 = ws[0]  # [64,3,11,11]
    w0s = np.zeros((3, 3, 48, 64), np.float32)
    for kqy in range(3):
        for kqx in range(3):
            for ry in range(4):
                for rx in range(4):
                    ky, kx = 4 * kqy + ry, 4 * kqx + rx
                    if ky <= 10 and kx <= 10:
                        for c in range(3):
                            w0s[kqy, kqx, c * 16 + ry * 4 + rx, :] = w0[:, c, ky, kx]
    # w0 dup on partitions 0-47 and 64-111 handled by two DMAs of same param
    # L1: [128(dup k),25,128]
    w1 = ws[1]  # [128,64,5,5]
    w1l = np.transpose(w1, (1, 2, 3, 0)).reshape(64, 25, 128).astype(np.float32)
    w1d = np.concatenate([w1l, w1l], axis=0)  # [128,25,128]
    # L2: [128,9,128]
    w2l = np.transpose(ws[2], (1, 2, 3, 0)).reshape(128, 9, 128).astype(np.float32)
    # L3: [128,9,2,128]
    w3 = ws[3].reshape(2, 128, 128, 3, 3)  # [h,m,k,ky,kx]
    w3l = np.transpose(w3, (2, 3, 4, 0, 1)).reshape(128, 9, 2, 128).astype(np.float32)

    # x -> pad 5 -> s2d [B,T,48,75,75]
    xp = np.zeros((B, T, 3, 300, 300), np.float32)
    xp[:, :, :, 5:293, 5:293] = x
    xs = xp.reshape(B, T, 3, 75, 4, 75, 4)          # b t c yq ry xq rx
    xs = np.transpose(xs, (0, 1, 2, 4, 6, 3, 5))    # b t c ry rx yq xq
    xs = xs.reshape(B, T, 48, 75, 75).copy()

    th0p = np.concatenate([th_pc[0], th_pc[0]]).reshape(128, 1)
    th1p = th_pc[1].reshape(128, 1)
    th2p = th_pc[2].reshape(128, 1)
    th3p = th_pc[3].reshape(2, 128).T.copy()  # [p, h]

    in_maps = []
    for core in range(N_CORES):
        in_maps.append({
            'xs': xs[2 * core: 2 * core + 2],
            'w0': w0s, 'w1': w1d, 'w2': w2l, 'w3': w3l,
            'th0p': th0p, 'th1p': th1p, 'th2p': th2p, 'th3p': th3p,
        })
    return in_maps


def build_nc(repeat=1, f32r_layers=(True, True, True, True)):
    import concourse.bacc as bacc
    import concourse.mybir as mybir
    from concourse import tile

    f32 = mybir.dt.float32
    f32r = mybir.dt.float32r
    AT = mybir.AluOpType
    dt_l = [f32r if u else f32 for u in f32r_layers]

    nc = bacc.Bacc("TRN2", target_bir_lowering=False, debug=False)
    xs_d = nc.declare_dram_parameter("xs", [2, T, 48, S2D, S2D], dt_l[0], isOutput=False)
    w0_d = nc.declare_dram_parameter("w0", [3, 3, 48, 64], dt_l[0], isOutput=False)
    w1_d = nc.declare_dram_parameter("w1", [128, 25, 128], dt_l[1], isOutput=False)
    w2_d = nc.declare_dram_parameter("w2", [128, 9, 128], dt_l[2], isOutput=False)
    w3_d = nc.declare_dram_parameter("w3", [128, 9, 2, 128], dt_l[3], isOutput=False)
    th_ds = [nc.declare_dram_parameter(f"th{i}p", [128, 2 if i == 3 else 1], f32,
                                       isOutput=False) for i in range(4)]
    out_d = nc.declare_dram_parameter("out", [2, 256, H3, H3], f32, isOutput=True)

    with tile.TileContext(nc) as tc:
        with (
            tc.tile_pool(name="const", bufs=1) as cpool,
            tc.tile_pool(name="state", bufs=1) as spool,
            tc.tile_pool(name="xin", bufs=2) as xpool,
            tc.tile_pool(name="ps", bufs=6, space="PSUM") as pspool,
        ):
            w0sb = cpool.tile([128, 3, 3, 64], dt_l[0])
            w1sb = cpool.tile([128, 25, 128], dt_l[1])
            w2sb = cpool.tile([128, 9, 128], dt_l[2])
            w3sb = cpool.tile([128, 9, 2, 128], dt_l[3])
            nc.sync.dma_start(w0sb[0:48], w0_d.ap().rearrange("a b k m -> k a b m"))
            nc.sync.dma_start(w0sb[64:112], w0_d.ap().rearrange("a b k m -> k a b m"))
            nc.sync.dma_start(w1sb[:], w1_d[:])
            nc.sync.dma_start(w2sb[:], w2_d[:])
            nc.sync.dma_start(w3sb[:], w3_d[:])
            thp = [cpool.tile([128, 2 if i == 3 else 1], f32, name=f"thp{i}")
                   for i in range(4)]
            for i in range(4):
                nc.sync.dma_start(thp[i][:], th_ds[i][:])

            plane0 = spool.tile([128, P0, P0], dt_l[1])
            plane1 = [spool.tile([128, P12, P12], dt_l[2], name=f"plane1_{s}") for s in (0, 1)]
            plane2 = [spool.tile([128, P12, P12], dt_l[3], name=f"plane2_{s}") for s in (0, 1)]
            mem0 = [spool.tile([64, H0 * H0], f32, name=f"mem0_{s}") for s in (0, 1)]
            mem1 = [spool.tile([128, H1 * H1], f32, name=f"mem1_{s}") for s in (0, 1)]
            mem2 = [spool.tile([128, H1 * H1], f32, name=f"mem2_{s}") for s in (0, 1)]
            mem3 = [spool.tile([128, 2 * H3 * H3], f32, name=f"mem3_{s}") for s in (0, 1)]
            acc = [spool.tile([128, 2 * H3 * H3], f32, name=f"acc_{s}") for s in (0, 1)]
            sp3 = [spool.tile([128, H3 * H3], f32, name=f"sp3_{s}") for s in (0, 1)]

            nc.gpsimd.memset(plane0.bitcast(f32)[:], 0.0)
            for s in (0, 1):
                nc.gpsimd.memset(plane1[s].bitcast(f32)[:], 0.0)
                nc.gpsimd.memset(plane2[s].bitcast(f32)[:], 0.0)

            OFF9 = [(ky, kx) for ky in range(3) for kx in range(3)]
            OFF25 = [(ky, kx) for ky in range(5) for kx in range(5)]

            def lif(mem_sl, th_ap, ps_ap, sp_out):
                # mem = mem*tau + psum ; sp = mem > th ; mem = (mem <= th) * mem
                nc.vector.scalar_tensor_tensor(mem_sl, mem_sl, TAU, ps_ap,
                                               AT.mult, AT.add)
                nc.vector.tensor_scalar(sp_out, mem_sl, th_ap, None, AT.is_gt)
                nc.vector.scalar_tensor_tensor(mem_sl, mem_sl, th_ap, mem_sl,
                                               AT.is_le, AT.mult)

            for rep in range(repeat):
                for s in (0, 1):
                    nc.vector.memset(mem0[s][:], 0.0)
                    nc.vector.memset(mem1[s][:], 0.0)
                    nc.vector.memset(mem2[s][:], 0.0)
                    nc.vector.memset(mem3[s][:], 0.0)
                    nc.vector.memset(acc[s][:], 0.0)

                for t in range(T):
                    # ---------------- L0: s2d 48ch 3x3 s1, both samples row/col-split
                    xt = xpool.tile([128, S2D, S2D], dt_l[0], name="xt")
                    nc.sync.dma_start(xt[0:48], xs_d[0, t])
                    nc.sync.dma_start(xt[64:112], xs_d[1, t])
                    for n in range(NT0):
                        ps0 = [pspool.tile([64, 432], f32, name=f"ps0_{s}", tag="ps")
                               for s in (0, 1)]
                        for o, (ky, kx) in enumerate(OFF9):
                            for s in (0, 1):
                                rb = 64 * s
                                nc.tensor.matmul(
                                    ps0[s][:],
                                    w0sb[rb:rb + 48, ky, kx, :],
                                    xt[rb:rb + 48, 6 * n + ky: 6 * n + ky + 6,
                                       kx: kx + 72],
                                    start=(o == 0), stop=(o == 8),
                                    tile_position=(rb, 0))
                        sl = np.s_[:, 432 * n: 432 * (n + 1)]
                        for s in (0, 1):
                            lif(mem0[s][sl], thp[0][0:64, 0:1], ps0[s][:],
                                plane0[64 * s: 64 * s + 64,
                                       2 + 6 * n: 8 + 6 * n, 2: 74])
                    # ---------------- L1: 64ch 5x5 s2, samples on row-groups
                    p0r = plane0.rearrange("p (y a) (x b) -> p y a x b", a=2, b=2)
                    for n in range(NT12):
                        psl = [pspool.tile([128, 432], f32, name=f"ps1_{s}", tag="ps") for s in (0, 1)]
                        for o, (ky, kx) in enumerate(OFF25):
                            kyq, kyr = divmod(ky, 2)
                            kxq, kxr = divmod(kx, 2)
                            for s in (0, 1):
                                rb = 64 * s
                                nc.tensor.matmul(
                                    psl[s][:],
                                    w1sb[rb:rb + 64, o, :],
                                    p0r[rb:rb + 64, 12 * n + kyq: 12 * n + kyq + 12,
                                        kyr, kxq: kxq + 36, kxr],
                                    start=(o == 0), stop=(o == 24),
                                    tile_position=(rb, 0))
                        for s in (0, 1):
                            sl = np.s_[:, 432 * n: 432 * (n + 1)]
                            lif(mem1[s][sl], thp[1][:, 0:1], psl[s][:],
                                plane1[s][:, 1 + 12 * n: 13 + 12 * n, 1: 37])
                    # ---------------- L2: 128ch 3x3 s1
                    for s in (0, 1):
                        for n in range(NT12):
                            ps = pspool.tile([128, 432], f32, name="ps2", tag="ps")
                            for o, (ky, kx) in enumerate(OFF9):
                                nc.tensor.matmul(
                                    ps[:], w2sb[:, o, :],
                                    plane1[s][:, 12 * n + ky: 12 * n + ky + 12,
                                              kx: kx + 36],
                                    start=(o == 0), stop=(o == 8))
                            sl = np.s_[:, 432 * n: 432 * (n + 1)]
                            lif(mem2[s][sl], thp[2][:, 0:1], ps[:],
                                plane2[s][:, 1 + 12 * n: 13 + 12 * n, 1: 37])
                    # ---------------- L3: 128ch 3x3 s2, 256 out = 2 halves
                    for s in (0, 1):
                        p2r = plane2[s].rearrange("p (y a) (x b) -> p y a x b", a=2, b=2)
                        for h in (0, 1):
                            ps = pspool.tile([128, 324], f32, name="ps3", tag="ps")
                            for o, (ky, kx) in enumerate(OFF9):
                                kyq, kyr = divmod(ky, 2)
                                kxq, kxr = divmod(kx, 2)
                                nc.tensor.matmul(
                                    ps[:], w3sb[:, o, h, :],
                                    p2r[:, kyq: kyq + 18, kyr, kxq: kxq + 18, kxr],
                                    start=(o == 0), stop=(o == 8))
                            sl = np.s_[:, 324 * h: 324 * (h + 1)]
                            lif(mem3[s][sl], thp[3][:, h:h + 1], ps[:], sp3[s][:])
                            nc.vector.tensor_tensor(acc[s][sl], acc[s][sl],
                                                    sp3[s][:], AT.add)
                # mean over T and store
                for s in (0, 1):
                    nc.vector.tensor_scalar(acc[s][:], acc[s][:], 1.0 / T, None,
                                            AT.mult)
                    for h in (0, 1):
                        nc.sync.dma_start(out_d[s, 128 * h: 128 * (h + 1)],
                                          acc[s][:, 324 * h: 324 * (h + 1)])

    nc.compile()
    return nc


F32R_LAYERS = (False, False, False, False)


def get_nc(repeat=1):
    key = ('nc', repeat, F32R_LAYERS)
    if key not in _CACHE:
        _CACHE[key] = build_nc(repeat, F32R_LAYERS)
    return _CACHE[key]


def kernel(**inputs):
    from concourse.bass_utils import run_bass_kernel_spmd
    nc = get_nc(repeat=1)
    in_maps = host_prep(inputs)
    res = run_bass_kernel_spmd(nc, in_maps, core_ids=list(range(N_CORES)))
    out = np.concatenate([res.results[c]["out"] for c in range(N_CORES)], axis=0)
    return out.astype(np.float32)
